# revision 1
# baseline (speedup 1.0000x reference)
"""Trainium2 Bass kernel for AdaptSelfAttention (Transformer-XL style relative
position attention).

Shapes (hardcoded): B=4, L=1024, H=512, NH=8, HD=64.
Sharding: 32 (batch, head) pairs -> 8 cores; core c handles batch c//2 and the
4-head group c%2 (hidden slice of 256 columns).

Math per (b, n):
  q = query @ Wq + bq   (per-head slice)          [L, 64]
  v = value @ Wv + bv                              [L, 64]
  k = key slice                                    [L, 64]
  rel = emb @ Wr + br                              [2L, 64]  (emb = sinusoid const)
  S[q_,k_] = (q+rrb).k  +  (q+rwb).rel[L+k_-q_]  +  k.rel[L+q_-k_]  + c2[k_]
       (c2[k_] = k.br ; the q-side br term is constant per row -> softmax-
        invariant, dropped)
  out = softmax_k(S with key-mask) @ v

Kernel computes S^T tiles (k on partitions, q free):
  - AC^T via matmul (contraction d=64)
  - E-term: E_[k,l] = k.rel[l] tiles -> DRAM -> skewed (diagonal-AP) DMA re-read
    gives E_sh^T directly (bf16), DVE-added pre-exp
  - BD-term: B_[q,l] = (q+rwb).rel[l] tiles -> DRAM -> skew read (cast to f32)
    gives BD_sh (S-orientation) -> PE transpose-accumulate into the f32 S PSUM
  - c2 + key-mask enter as the per-partition bias of the exp() activation
  - AV: lhsT = [v | 1] so the softmax denominator is row 64 of the output PSUM
Pairs are software-pipelined: pair p+1's B_/E_ production is emitted between
pair p's skew reads and its ki-loop.
"""

import math
import sys

import numpy as np

sys.path.insert(0, "/opt/trn_rl_repo")

import concourse.bass as bass
import concourse.tile as tile
from concourse.tile_rust import add_dep_helper
from concourse import bacc, mybir
from concourse.bass_utils import run_bass_kernel_spmd

import ml_dtypes

BF16 = ml_dtypes.bfloat16

B, L, H, NH, HD = 4, 1024, 512, 8, 64
PITCH = 1152  # stored l-window width per row of the B_/E_ scratch
NEG = -1e30


def _get_embedding(max_len, dim):
    half = dim // 2
    freq = np.exp(np.arange(half, dtype=np.float64) * (-math.log(10000.0) / (half - 1)))
    pos = np.arange(-max_len, max_len, dtype=np.float64)
    ang = pos[:, None] * freq[None, :]
    return np.concatenate([np.sin(ang), np.cos(ang)], axis=1)


def build_body(tc, ins, outs):
    """Emit the per-core kernel. ins/outs: dicts of bass.AP over DRAM."""
    nc = tc.nc
    f32 = mybir.dt.float32
    bf16 = mybir.dt.bfloat16
    Ident = mybir.ActivationFunctionType.Identity
    Exp = mybir.ActivationFunctionType.Exp

    from contextlib import ExitStack

    ctx = ExitStack()
    with ctx:
        # ---- pools ----
        io = ctx.enter_context(tc.tile_pool(name="io", bufs=1))
        persist = ctx.enter_context(tc.tile_pool(name="persist", bufs=1))
        bdp = ctx.enter_context(tc.tile_pool(name="bdp", bufs=3))    # bf16 batches
        ep = ctx.enter_context(tc.tile_pool(name="ep", bufs=3))      # esh batches
        pp = ctx.enter_context(tc.tile_pool(name="pp", bufs=6))      # exp outputs
        stg = ctx.enter_context(tc.tile_pool(name="stg", bufs=8))
        sml = ctx.enter_context(tc.tile_pool(name="sml", bufs=4))
        # PSUM: stagePS 2x[128,512]=2; psS f32 2; psB bf16 2x1=2; psO 2 -> 8
        stagePS = ctx.enter_context(tc.tile_pool(name="stagePS", bufs=2, space="PSUM"))
        psS = ctx.enter_context(tc.tile_pool(name="psS", bufs=1, space="PSUM"))
        psB = ctx.enter_context(tc.tile_pool(name="psB", bufs=2, space="PSUM"))
        psO = ctx.enter_context(tc.tile_pool(name="psO", bufs=1, space="PSUM"))
        dscr = ctx.enter_context(tc.tile_pool(name="dscr", bufs=2, space="DRAM"))
        descr = ctx.enter_context(tc.tile_pool(name="descr", bufs=2, space="DRAM"))

        # ---- stage constant/weight inputs into SBUF ----
        ident = persist.tile([128, 128], bf16, tag="ident")
        nc.scalar.dma_start(ident[:], ins["ident_bf"])
        identf = persist.tile([128, 128], f32, tag="identf")
        nc.scalar.dma_start(identf[:], ins["ident_f32"])

        relT = persist.tile([128, 2056], bf16, tag="relT")
        nc.scalar.dma_start(relT[:, 0:2049], ins["relTa"])

        # Wq/Wv [512, 256] -> [128, 4*256]
        wq_sb = persist.tile([128, 1024], bf16, tag="wq")
        wv_sb = persist.tile([128, 1024], bf16, tag="wv")
        for k in range(4):
            nc.scalar.dma_start(wq_sb[:, k * 256:(k + 1) * 256],
                                ins["Wq"][k * 128:(k + 1) * 128, :])
            nc.scalar.dma_start(wv_sb[:, k * 256:(k + 1) * 256],
                                ins["Wv"][k * 128:(k + 1) * 128, :])
        # qT/vT [512, 1024] -> 4 tiles each
        qT_sb, vT_sb = [], []
        for k in range(4):
            t = io.tile([128, 1024], bf16, tag=f"qT{k}", name=f"qTs{k}")
            nc.scalar.dma_start(t[:], ins["qT"][k * 128:(k + 1) * 128, :])
            qT_sb.append(t)
        for k in range(4):
            t = io.tile([128, 1024], bf16, tag=f"vT{k}", name=f"vTs{k}")
            nc.scalar.dma_start(t[:], ins["vT"][k * 128:(k + 1) * 128, :])
            vT_sb.append(t)
        # kT [256, 1024] -> 2 tiles
        kT_sb = []
        for t_ in range(2):
            t = persist.tile([128, 1024], bf16, tag=f"kT{t_}", name=f"kTs{t_}")
            nc.scalar.dma_start(t[:], ins["kT"][t_ * 128:(t_ + 1) * 128, :])
            kT_sb.append(t)
        # small vectors
        bq_sb = sml.tile([128, 2], f32, tag="bq")
        nc.scalar.dma_start(bq_sb[:], ins["bq2"])
        brr_sb = sml.tile([128, 2], f32, tag="brr")
        nc.scalar.dma_start(brr_sb[:], ins["brr2"])
        brw_sb = sml.tile([128, 2], f32, tag="brw")
        nc.scalar.dma_start(brw_sb[:], ins["brw2"])
        mb_sb = sml.tile([128, 8], f32, tag="mb")
        nc.scalar.dma_start(mb_sb[:], ins["maskbias"])
        bv_sb = sml.tile([128, 256], f32, tag="bv")
        nc.scalar.dma_start(bv_sb[:], ins["bv128"])

        biasA = sml.tile([128, 2], f32, tag="biasA")  # bq + r_r_bias
        nc.vector.tensor_add(biasA[:], bq_sb[:], brr_sb[:])
        biasB = sml.tile([128, 2], f32, tag="biasB")  # bq + r_w_bias
        nc.vector.tensor_add(biasB[:], bq_sb[:], brw_sb[:])

        # ---- q projection: qrrT/grwT [2 x (128, 1024)] (d on partitions) ----
        qrrT = [persist.tile([128, 1024], bf16, tag=f"qrrT{i}", name=f"qrrT{i}")
                for i in range(2)]
        grwT = [persist.tile([128, 1024], bf16, tag=f"grwT{i}", name=f"grwT{i}")
                for i in range(2)]
        for t_ in range(2):
            for nh in range(2):
                ps = stagePS.tile([128, 512], f32, tag="ps")
                for k in range(4):
                    nc.tensor.matmul(
                        ps[:],
                        wq_sb[:, k * 256 + t_ * 128: k * 256 + (t_ + 1) * 128],
                        qT_sb[k][:, nh * 512:(nh + 1) * 512],
                        start=(k == 0), stop=(k == 3),
                    )
                nc.scalar.activation(qrrT[t_][:, nh * 512:(nh + 1) * 512], ps[:],
                                     Ident, bias=biasA[:, t_:t_ + 1], scale=1.0)
                nc.scalar.activation(grwT[t_][:, nh * 512:(nh + 1) * 512], ps[:],
                                     Ident, bias=biasB[:, t_:t_ + 1], scale=1.0)

        # ---- v projection -> v_sb tiles [128, 4*65] ([v_head | 1]) ----
        v_sb = []
        for lt in range(8):
            ps = stagePS.tile([128, 512], f32, tag="ps")
            for k in range(4):
                nc.tensor.matmul(
                    ps[:, 0:256],
                    vT_sb[k][:, lt * 128:(lt + 1) * 128],
                    wv_sb[:, k * 256:(k + 1) * 256],
                    start=(k == 0), stop=(k == 3),
                )
            nc.vector.tensor_add(ps[:, 0:256], ps[:, 0:256], bv_sb[:])
            vt = persist.tile([128, 260], bf16, tag=f"vsb{lt}", name=f"vsb{lt}")
            src = ps[:, 0:256].rearrange("p (h d) -> p h d", d=64)
            dst = vt[:].rearrange("p (h e) -> p h e", e=65)[:, :, 0:64]
            nc.vector.tensor_copy(dst, src)
            nc.vector.memset(vt[:].rearrange("p (h e) -> p h e", e=65)[:, :, 64:65], 1.0)
            v_sb.append(vt)

        copy_engines = [
            lambda o_, i_: nc.scalar.copy(o_, i_),
            lambda o_, i_: nc.vector.tensor_copy(o_, i_),
        ]

        # ---- per-(b,head) pair loop, software-pipelined ----
        pair_state = {}

        def produce_init(p):
            scrB = dscr.tile([1024, PITCH], bf16, tag="scrB", name=f"scrB{p}")
            scrE = descr.tile([1024, PITCH], bf16, tag="scrE", name=f"scrE{p}")
            c2 = sml.tile([128, 8], f32, tag="c2", name=f"c2_{p}")
            ebias = sml.tile([128, 8], f32, tag="ebias", name=f"ebias{p}")
            pair_state[p] = (scrB, scrE, c2, ebias)

        def produce_tile(p, src_sel, qi):
            """One B_ (src_sel=0) or E_ (1) tile of pair p -> DRAM scratch."""
            t_ = p // 2
            o = (p % 2) * 64
            scrB, scrE, c2, ebias = pair_state[p]
            scr = scrB if src_sel == 0 else scrE
            W0 = 897 - 128 * qi
            if src_sel == 0:
                lhs = grwT[t_][o:o + 64, qi * 128:(qi + 1) * 128]
            else:
                lhs = kT_sb[t_][o:o + 64, qi * 128:(qi + 1) * 128]
            sb = stg.tile([128, PITCH], bf16, tag="stg")
            for ci, (c0, cw) in enumerate(((0, 512), (512, 512), (1024, 128))):
                ps = stagePS.tile([128, 512], f32, tag="ps")
                nc.tensor.matmul(ps[:, 0:cw], lhs,
                                 relT[o:o + 64, W0 + c0:W0 + c0 + cw],
                                 start=True, stop=True)
                if src_sel == 1 and ci == 2:
                    nc.tensor.matmul(ps[:, 128:129], lhs,
                                     relT[o:o + 64, 2048:2049],
                                     start=True, stop=True)
                    nc.scalar.activation(c2[:, qi:qi + 1], ps[:, 128:129],
                                         Ident, bias=0.0, scale=1.0)
                eng = copy_engines[0 if (qi * 3 + ci) % 3 == 0 else 1]
                eng(sb[:, c0:c0 + cw], ps[:, 0:cw])
            nc.sync.dma_start(scr[qi * 128:(qi + 1) * 128, :], sb[:])

        def produce_fini(p):
            scrB, scrE, c2, ebias = pair_state[p]
            nc.vector.tensor_add(ebias[:], c2[:], mb_sb[:])

        def skew_reads(p):
            """Batched diagonal re-reads for pair p."""
            scrB, scrE, c2, ebias = pair_state[p]
            scrB_ap, scrE_ap = scrB[:], scrE[:]
            # BD (bf16, qi-major): [128, 8*1024]
            bdall = bdp.tile([128, 8192], bf16, tag="bd", name=f"bdall{p}")
            srcB = bass.AP(scrB_ap.tensor, scrB_ap.offset + 127,
                           [[PITCH - 1, 128], [128 * PITCH, 8], [1, 1024]])
            nc.sync.dma_start(bdall[:].rearrange("p (a b) -> p a b", a=8), srcB)
            # E (bf16, ki-major by construction)
            eshall = ep.tile([128, 8192], bf16, tag="esh", name=f"esh{p}")
            srcE = bass.AP(scrE_ap.tensor, scrE_ap.offset + 127,
                           [[PITCH - 1, 128], [128 * PITCH, 8], [1, 1024]])
            nc.sync.dma_start(eshall[:].rearrange("p (a b) -> p a b", a=8), srcE)
            return bdall, eshall

        def ki_iter(p, ki, bdall, eshall, outT):
            t_ = p // 2
            o = (p % 2) * 64
            kTs = kT_sb[t_]
            qrr = qrrT[t_]
            ebias = pair_state[p][3]
            BDT = psB.tile([128, 1024], bf16, tag="bdt")
            for qi in range(8):
                nc.tensor.matmul(
                    BDT[:, qi * 128:(qi + 1) * 128],
                    bdall[:, qi * 1024 + ki * 128: qi * 1024 + (ki + 1) * 128],
                    ident[:],
                    is_transpose=True, start=True, stop=True,
                )
            X = pp.tile([128, 1024], bf16, tag="y")
            nc.vector.tensor_add(X[:], BDT[:],
                                 eshall[:, ki * 1024:(ki + 1) * 1024])
            ST = psS.tile([128, 1024], f32, tag="s")
            for nh in range(2):
                nc.tensor.matmul(
                    ST[:, nh * 512:(nh + 1) * 512],
                    kTs[o:o + 64, ki * 128:(ki + 1) * 128],
                    qrr[o:o + 64, nh * 512:(nh + 1) * 512],
                    start=True, stop=True,
                )
            nc.vector.tensor_add(ST[:], ST[:], X[:])
            P = pp.tile([128, 1024], bf16, tag="p")
            nc.scalar.activation(P[:], ST[:], Exp,
                                 bias=ebias[:, ki:ki + 1], scale=1.0)
            for nh in range(2):
                nc.tensor.matmul(
                    outT[0:65, nh * 512:(nh + 1) * 512],
                    v_sb[ki][:, p * 65:p * 65 + 65],
                    P[:, nh * 512:(nh + 1) * 512],
                    start=(ki == 0), stop=(ki == 7),
                )

        def finalize(p, outT):
            oT = sml.tile([65, 1024], f32, tag="oT", name=f"oT{p}")
            nc.scalar.activation(oT[:], outT[:], Ident, bias=0.0, scale=1.0)
            for qi in range(8):
                psF = stagePS.tile([128, 512], f32, tag="ps")
                nc.tensor.matmul(psF[0:128, 0:65],
                                 oT[0:65, qi * 128:(qi + 1) * 128],
                                 identf[0:65, 0:65],
                                 is_transpose=True, start=True, stop=True)
                rec = sml.tile([128, 1], f32, tag="rec")
                nc.vector.reciprocal(rec[:], psF[:, 64:65])
                fin = sml.tile([128, 64], f32, tag="fin")
                nc.vector.tensor_scalar_mul(fin[:], psF[:, 0:64], rec[:, 0:1])
                nc.sync.dma_start(outs["out"][p, qi * 128:(qi + 1) * 128, :], fin[:])

        def produce(p):
            produce_init(p)
            for src_sel in range(2):
                for qi in range(8):
                    produce_tile(p, src_sel, qi)
            produce_fini(p)

        produce(0)
        reads = {0: skew_reads(0)}
        for p in range(4):
            bdall, eshall = reads.pop(p)
            if p < 3:
                produce(p + 1)
                reads[p + 1] = skew_reads(p + 1)
            outT = psO.tile([65, 1024], f32, tag="o", name=f"outT{p}")
            for ki in range(8):
                ki_iter(p, ki, bdall, eshall, outT)
            finalize(p, outT)


_CACHE = {}


def _build_nc():
    if "nc" in _CACHE:
        return _CACHE["nc"]
    nc = bacc.Bacc("TRN2", target_bir_lowering=False, debug=False,
                   enable_asserts=False, num_devices=8)
    f32 = mybir.dt.float32
    bf16 = mybir.dt.bfloat16
    ins = {}

    def di(name, shape, dt):
        ins[name] = nc.dram_tensor(name, shape, dt, kind="ExternalInput").ap()

    di("qT", [512, 1024], bf16)
    di("vT", [512, 1024], bf16)
    di("kT", [256, 1024], bf16)
    di("Wq", [512, 256], bf16)
    di("Wv", [512, 256], bf16)
    di("relTa", [128, 2049], bf16)
    di("bq2", [128, 2], f32)
    di("brr2", [128, 2], f32)
    di("brw2", [128, 2], f32)
    di("maskbias", [128, 8], f32)
    di("bv128", [128, 256], f32)
    di("ident_bf", [128, 128], bf16)
    di("ident_f32", [128, 128], f32)
    outs = {"out": nc.dram_tensor("out", [4, 1024, 64], f32, kind="ExternalOutput").ap()}

    with tile.TileContext(nc) as tc:
        build_body(tc, ins, outs)
    nc.compile()
    _CACHE["nc"] = nc
    return nc


def make_in_maps(query, key, value, w_q_w, w_q_b, w_v_w, w_v_b, w_r_w, w_r_b,
                 r_r_bias, r_w_bias, seq_len):
    emb = _get_embedding(L, H)
    rel = (emb @ w_r_w.astype(np.float64) + w_r_b.astype(np.float64))  # [2L, 64]
    relTa = np.zeros((128, 2049), dtype=BF16)
    relTa[0:64, 0:2048] = rel.T.astype(BF16)
    relTa[0:64, 2048] = w_r_b.astype(BF16)
    relTa[64:128, :] = relTa[0:64, :]

    ident_bf = np.eye(128, dtype=BF16)
    ident_f32 = np.eye(128, dtype=np.float32)
    seq_len = int(seq_len)
    in_maps = []
    for c in range(8):
        b, hg = c // 2, c % 2
        hs = 256 * hg
        heads = slice(4 * hg, 4 * hg + 4)
        mb = np.where((np.arange(1024) < seq_len), 0.0, NEG).astype(np.float32)
        in_maps.append({
            "qT": np.ascontiguousarray(query[b].T).astype(BF16),
            "vT": np.ascontiguousarray(value[b].T).astype(BF16),
            "kT": np.ascontiguousarray(key[b][:, hs:hs + 256].T).astype(BF16),
            "Wq": np.ascontiguousarray(w_q_w[:, hs:hs + 256]).astype(BF16),
            "Wv": np.ascontiguousarray(w_v_w[:, hs:hs + 256]).astype(BF16),
            "relTa": relTa,
            "bq2": np.ascontiguousarray(w_q_b[hs:hs + 256].reshape(2, 128).T).astype(np.float32),
            "brr2": np.ascontiguousarray(r_r_bias[heads].reshape(2, 128).T).astype(np.float32),
            "brw2": np.ascontiguousarray(r_w_bias[heads].reshape(2, 128).T).astype(np.float32),
            "maskbias": np.ascontiguousarray(mb.reshape(8, 128).T).astype(np.float32),
            "bv128": np.tile(w_v_b[hs:hs + 256][None, :], (128, 1)).astype(np.float32),
            "ident_bf": ident_bf,
            "ident_f32": ident_f32,
        })
    return in_maps


def kernel(query, key, value, w_q_w, w_q_b, w_v_w, w_v_b, w_r_w, w_r_b,
           r_r_bias, r_w_bias, seq_len, _trace=False):
    query = np.asarray(query); key = np.asarray(key); value = np.asarray(value)
    w_q_w = np.asarray(w_q_w); w_q_b = np.asarray(w_q_b)
    w_v_w = np.asarray(w_v_w); w_v_b = np.asarray(w_v_b)
    w_r_w = np.asarray(w_r_w); w_r_b = np.asarray(w_r_b)
    r_r_bias = np.asarray(r_r_bias); r_w_bias = np.asarray(r_w_bias)

    nc = _build_nc()
    in_maps = make_in_maps(query, key, value, w_q_w, w_q_b, w_v_w, w_v_b,
                           w_r_w, w_r_b, r_r_bias, r_w_bias, seq_len)
    res = run_bass_kernel_spmd(nc, in_maps, core_ids=list(range(8)), trace=_trace)
    out = np.zeros((B, L, H), dtype=np.float32)
    for c in range(8):
        b, hg = c // 2, c % 2
        o = res.results[c]["out"]  # [4, 1024, 64]
        for j in range(4):
            out[b][:, 256 * hg + 64 * j: 256 * hg + 64 * (j + 1)] = o[j]
    if _trace:
        return out, res
    return out

